# revision 11
# baseline (speedup 1.0000x reference)
"""nn_AttentionOnDetail Trainium2 Bass kernel, 8 NeuronCores.

Sharding: stage 1 (AFT) is T-sharded (each core owns 256 timesteps of both
batches); AllToAll #1 re-shards to head-parallel (each core owns 2 heads of
16) for the causal SDPA; AllToAll #2 re-shards back to T for the output
projection.  Per-core layout is (128 partitions, 4 row-tiles, 1024 ch) with
row r = rt*128 + p  (rt 0,1 = batch 0; rt 2,3 = batch 1).

Math notes:
 - xs = 2*pi*sigmoid(x) - pi == pi*tanh(x/2), so the AFT harmonics are
   sin/cos(n*pi*h), h = tanh(x/2), built from s=sin(pi*h), c=cos(pi*h)
   via Chebyshev products; cos(pi*h) = 1 - 2*sin(pi*h/2)^2 keeps every
   ACT Sin argument inside [-pi, pi] (the HW spline is garbage beyond).
 - The 3x9 harmonic combination runs on the TensorEngine as 24 fp32r
   matmuls with scaled-identity stationary weights (PSUM accumulates).
 - rms_norm rsqrt runs on DVE (bit-trick seed + 2 Newton steps).
 - SDPA: q == k (reference feeds the same tensor through _process_qkv),
   scores are computed transposed (S^T[k,q]) so exp(PSUM)->fp16 feeds
   attn@V directly as matmul rhs with V' = [V | 1] stationary; the
   appended ones column yields softmax row-sums for free.  No
   max-subtraction needed: |scores*0.12| <= 7.7.
"""
import sys
import numpy as np

sys.path.insert(0, "/opt/trn_rl_repo")

import concourse.bass as bass
import concourse.mybir as mybir
import concourse.tile as tile
from concourse import bacc
from concourse.bass_utils import run_bass_kernel_spmd
from concourse.masks import make_identity

F32 = mybir.dt.float32
F32R = mybir.dt.float32r
F16 = mybir.dt.float16
I32 = mybir.dt.int32
AF = mybir.ActivationFunctionType
ALU = mybir.AluOpType

B, T, C, H, HD = 2, 2048, 1024, 16, 64
NCORES = 8
TSH = T // NCORES            # 256 timesteps per core
R = B * TSH                  # 512 rows per core
EPS = 1.1920929e-07
SDPA_SCALE = 0.12
PI = float(np.pi)

_CACHE = {}


def _quake_rsqrt(nc, pool, m_ap, n, tag):
    """rfac = 1/sqrt(m) for m (128, n) positive, on DVE only."""
    sh = [128, n]
    it = pool.tile(sh, I32, tag=f"{tag}_i", name=f"{tag}_i")
    nc.vector.tensor_scalar(it[:], m_ap.bitcast(I32), 1, None,
                            ALU.logical_shift_right)
    sd = pool.tile(sh, I32, tag=f"{tag}_s", name=f"{tag}_s")
    nc.vector.tensor_scalar(sd[:], it[:], -1, 0x5F3759DF, ALU.mult, ALU.add)
    y0 = sd[:].bitcast(F32)
    t = pool.tile(sh, F32, tag=f"{tag}_t", name=f"{tag}_t")
    nc.vector.tensor_tensor(t[:], y0, y0, ALU.mult)
    nc.vector.tensor_tensor(t[:], t[:], m_ap, ALU.mult)
    nc.vector.tensor_scalar(t[:], t[:], -0.5, 1.5, ALU.mult, ALU.add)
    nc.vector.tensor_tensor(t[:], t[:], y0, ALU.mult)       # y1
    y2 = pool.tile(sh, F32, tag=f"{tag}_y", name=f"{tag}_y")
    nc.vector.tensor_tensor(y2[:], t[:], t[:], ALU.mult)
    nc.vector.tensor_tensor(y2[:], y2[:], m_ap, ALU.mult)
    nc.vector.tensor_scalar(y2[:], y2[:], -0.5, 1.5, ALU.mult, ALU.add)
    nc.vector.tensor_tensor(y2[:], y2[:], t[:], ALU.mult)
    return y2


def build():
    nc = bacc.Bacc("TRN2", target_bir_lowering=False, debug=False,
                   num_devices=NCORES)
    xs_d = nc.dram_tensor("xs", [128, 4, 1024], F32, kind="ExternalInput")
    combw_d = nc.dram_tensor("combw", [128, 24, 128], F32, kind="ExternalInput")
    kvec_d = nc.dram_tensor("kvec", [128, 4], F32, kind="ExternalInput")
    rotc1_d = nc.dram_tensor("rotc1", [128, 16, 16], F32, kind="ExternalInput")
    rots1_d = nc.dram_tensor("rots1", [128, 16, 16], F32, kind="ExternalInput")
    rotc2_d = nc.dram_tensor("rotc2", [128, 2, 16], F32, kind="ExternalInput")
    rots2_d = nc.dram_tensor("rots2", [128, 2, 16], F32, kind="ExternalInput")
    aftT_d = nc.dram_tensor("aftT", [128, 8, 1024], F32, kind="ExternalInput")
    mhaT_d = nc.dram_tensor("mhaT", [128, 8, 1024], F32, kind="ExternalInput")
    out_d = nc.dram_tensor("out", [128, 4, 1024], F32, kind="ExternalOutput")

    with tile.TileContext(nc) as tc:
      with tc.tile_pool(name="glob", bufs=1) as gp, \
           tc.tile_pool(name="dram", bufs=1, space="DRAM") as dpool:

        a2a1_in = [dpool.tile([NCORES * TSH, 128], F32, name=f"a2a1_in{b}")
                   for b in range(2)]
        a2a1_out = [dpool.tile([NCORES * TSH, 128], F32,
                               name=f"a2a1_out{b}") for b in range(2)]
        a2a2_in = [dpool.tile([NCORES * 128, 256], F32, name=f"a2a2_in{b}")
                   for b in range(2)]
        a2a2_out = [dpool.tile([NCORES * 128, 256], F32,
                               name=f"a2a2_out{b}") for b in range(2)]

        dum_in = dpool.tile([8, 4], F32, name="dum_in")
        dum_out = dpool.tile([8, 4], F32, name="dum_out")

        x = gp.tile([128, 4, 1024], F32, tag="bigA", name="x")
        nc.sync.dma_start(out=x[:], in_=xs_d[:])
        kvec = gp.tile([128, 4], F32, name="kvec")
        nc.sync.dma_start(out=kvec[:], in_=kvec_d[:])
        rotc1 = gp.tile([128, 16, 16], F32, name="rotc1")
        rots1 = gp.tile([128, 16, 16], F32, name="rots1")
        rotc2 = gp.tile([128, 2, 16], F32, name="rotc2")
        rots2 = gp.tile([128, 2, 16], F32, name="rots2")
        nc.sync.dma_start(out=rotc1[:], in_=rotc1_d[:])
        nc.sync.dma_start(out=rots1[:], in_=rots1_d[:])
        nc.sync.dma_start(out=rotc2[:], in_=rotc2_d[:])
        nc.sync.dma_start(out=rots2[:], in_=rots2_d[:])
        projw = gp.tile([128, 8, 1024], F32R, tag="projw", name="aftT")
        nc.sync.dma_start(out=projw[:], in_=aftT_d[:].bitcast(F32R))
        ident = gp.tile([128, 128], F32, name="ident")
        make_identity(nc, ident[:])
        nc.sync.dma_start(out=dum_in[:], in_=kvec_d[0:8, :])
        nc.gpsimd.collective_compute(
            "AllToAll", ALU.bypass,
            replica_groups=[list(range(NCORES))],
            ins=[dum_in[:].opt()], outs=[dum_out[:].opt()])

        qkv = []
        for nm in ("tq", "tk", "tv"):
            qkv.append(gp.tile([128, 4, 1024], F32, tag=nm, name=nm))
        ssqs = []
        for i3 in range(3):
            t3 = gp.tile([128, 64], F32, tag=f"ssq{i3}", name=f"ssq{i3}")
            ssqs.append(t3)

        # ---------------- stage 1 front-end + combine, per batch-half ----
        with nc.named_scope("fe_combine", notify=True), \
             tc.tile_pool(name="fe", bufs=1) as fep, \
             tc.tile_pool(name="psA", bufs=2, space="PSUM") as psA:
            combw = fep.tile([128, 24, 128], F32R, name="combw")
            nc.sync.dma_start(out=combw[:], in_=combw_d[:].bitcast(F32R))
            for half in range(2):
                xh = x[:].rearrange("p a b -> p (a b)")[:, 2048 * half:
                                                        2048 * (half + 1)]
                h = fep.tile([128, 2048], F32, tag="h", name="h")
                nc.scalar.activation(h[:], xh, AF.Tanh, scale=0.5)
                sA = fep.tile([128, 2048], F32R, tag="sA", name="sA")
                nc.scalar.activation(sA[:], h[:], AF.Sin, scale=PI)
                sB = fep.tile([128, 2048], F32, tag="sB", name="sB")
                nc.scalar.activation(sB[:], h[:], AF.Sin, scale=PI / 2)
                nc.scalar.activation(sB[:], sB[:], AF.Square)       # u
                c1 = fep.tile([128, 2048], F32R, tag="h", name="c1")
                nc.vector.tensor_scalar(c1[:], sB[:], -2.0, 1.0, ALU.mult,
                                        ALU.add)
                p_ = fep.tile([128, 2048], F32R, tag="p_", name="p_")
                nc.scalar.activation(p_[:], sA[:].bitcast(F32), AF.Square)
                m = fep.tile([128, 2048], F32R, tag="m", name="m")
                nc.vector.tensor_tensor(m[:], sA[:].bitcast(F32),
                                        c1[:].bitcast(F32), ALU.mult)
                sp = fep.tile([128, 2048], F32R, tag="sp", name="sp")
                nc.vector.tensor_tensor(sp[:], sA[:].bitcast(F32),
                                        p_[:].bitcast(F32), ALU.mult)
                cp = fep.tile([128, 2048], F32R, tag="cp", name="cp")
                nc.gpsimd.tensor_tensor(cp[:], c1[:].bitcast(F32),
                                        p_[:].bitcast(F32), ALU.mult)
                mp = fep.tile([128, 2048], F32R, tag="mp", name="mp")
                nc.gpsimd.tensor_tensor(mp[:], m[:].bitcast(F32),
                                        p_[:].bitcast(F32), ALU.mult)
                mm = fep.tile([128, 2048], F32R, tag="mm", name="mm")
                nc.scalar.activation(mm[:], m[:].bitcast(F32), AF.Square)
                basis = [sA, c1, m, p_, sp, cp, mp, mm]

                for chunk in range(4):
                    gofs = 2048 * half + 512 * chunk
                    rt, c0 = gofs // 1024, gofs % 1024
                    pss = []
                    for nm in ("cq", "ck", "cv"):
                        pst = psA.tile([128, 512], F32, tag=nm, name=nm)
                        pss.append(pst)
                    for f in range(8):
                        for i in range(3):
                            nc.tensor.matmul(
                                pss[i][:], combw[:, 8 * i + f, :],
                                basis[f][:, 512 * chunk:512 * (chunk + 1)],
                                start=(f == 0), stop=(f == 7))
                    dst = [qkv[i][:, rt, c0:c0 + 512] for i in range(3)]
                    nc.vector.tensor_scalar(dst[0], pss[0][:],
                                            kvec[:, 0:1], None, ALU.add)
                    nc.scalar.activation(dst[1], pss[1][:], AF.Identity,
                                         bias=kvec[:, 1:2])
                    nc.scalar.activation(dst[2], pss[2][:], AF.Identity,
                                         bias=kvec[:, 2:3])

                sqh = fep.tile([128, 2048], F32, tag="sqh", name="sqh")
                for i3 in range(3):
                    nc.scalar.activation(
                        sqh[:],
                        qkv[i3][:, 2 * half:2 * half + 2, :]
                        .rearrange("p a b -> p (a b)"), AF.Square)
                    nc.vector.tensor_reduce(
                        ssqs[i3][:, 32 * half:32 * (half + 1)],
                        sqh[:].rearrange("p (a h d) -> p (a h) d", a=2, h=16),
                        axis=mybir.AxisListType.X, op=ALU.add)

        # ---------------- stage 1 rms + rotary + AFT ---------------------
        with nc.named_scope("aft", notify=True), \
             tc.tile_pool(name="pB", bufs=1) as pB:
            rfs = []
            for i in range(3):
                ssq = ssqs[i]
                nc.vector.tensor_scalar(ssq[:], ssq[:], 1.0 / 64, EPS,
                                        ALU.mult, ALU.add)
                rfs.append(_quake_rsqrt(nc, pB, ssq[:], 64, f"rf{i}"))

            for i, eng in ((0, nc.vector), (1, nc.gpsimd), (2, nc.vector)):
                rf_b = rfs[i][:].rearrange("p (a h) -> p a h", a=4) \
                    .unsqueeze(3).broadcast_to([128, 4, 16, 64])
                v4 = qkv[i][:].rearrange("p a (h d) -> p a h d", h=16)
                eng.tensor_tensor(v4, v4, rf_b, ALU.mult)

            # rotary on q (DVE) and k (GPSIMD), in place, active quarter
            for i, eng in ((0, nc.vector), (1, nc.gpsimd)):
                qv = qkv[i][:].rearrange("p a (h d) -> p a h d", h=16)
                x1 = qv[:, :, :, 0:16]
                x2 = qv[:, :, :, 32:48]
                cb = rotc1[:].unsqueeze(1).broadcast_to([128, 4, 16, 16])
                sb_ = rots1[:].unsqueeze(1).broadcast_to([128, 4, 16, 16])
                u1 = pB.tile([128, 4, 16, 16], F32, tag=f"ru1{i}",
                             name=f"ru1{i}")
                u2 = pB.tile([128, 4, 16, 16], F32, tag=f"ru2{i}",
                             name=f"ru2{i}")
                t1 = pB.tile([128, 4, 16, 16], F32, tag=f"rt1{i}",
                             name=f"rt1{i}")
                eng.tensor_tensor(u1[:], x2, sb_, ALU.mult)
                eng.tensor_tensor(u2[:], x1, sb_, ALU.mult)
                eng.tensor_tensor(t1[:], x1, cb, ALU.mult)
                eng.tensor_tensor(x1, t1[:], u1[:], ALU.add)
                eng.tensor_tensor(t1[:], x2, cb, ALU.mult)
                eng.tensor_tensor(x2, t1[:], u2[:], ALU.subtract)

            # AFT attention: ek, s, r, y1  (pairing rt<->rt+2)
            ek = qkv[1]
            nc.scalar.activation(ek[:].rearrange("p a b -> p (a b)"),
                                 ek[:].rearrange("p a b -> p (a b)"), AF.Exp)
            s = pB.tile([128, 2, 1024], F32, tag="s_", name="s_")
            nc.vector.tensor_tensor(s[:], ek[:, 0:2, :], ek[:, 2:4, :],
                                    ALU.add)
            sv = pB.tile([128, 2, 1024], F32, tag="sv", name="sv")
            t_ = pB.tile([128, 2, 1024], F32, tag="t_", name="t_")
            nc.vector.tensor_tensor(sv[:], ek[:, 0:2, :], qkv[2][:, 0:2, :],
                                    ALU.mult)
            nc.gpsimd.tensor_tensor(t_[:], ek[:, 2:4, :], qkv[2][:, 2:4, :],
                                    ALU.mult)
            nc.vector.tensor_tensor(sv[:], sv[:], t_[:], ALU.add)
            # 1/s via ln+exp in place on s
            nc.scalar.activation(s[:], s[:], AF.Ln)
            nc.scalar.activation(s[:], s[:], AF.Exp, scale=-1.0)
            nc.vector.tensor_tensor(sv[:], sv[:], s[:], ALU.mult)   # r
            # y1 = (tanh(q/2)+1)*r   (0.5 folded into aftT)
            tq = qkv[0]
            nc.scalar.activation(tq[:].rearrange("p a b -> p (a b)"),
                                 tq[:].rearrange("p a b -> p (a b)"),
                                 AF.Tanh, scale=0.5)
            y1 = qkv[2]
            for b in range(2):
                ub = pB.tile([128, 2, 1024], F32, tag="t_", name=f"ub{b}")
                nc.vector.tensor_scalar(ub[:], tq[:, 2 * b:2 * b + 2, :],
                                        1.0, None, ALU.add)
                nc.vector.tensor_tensor(y1[:, 2 * b:2 * b + 2, :], ub[:],
                                        sv[:], ALU.mult)

        # ------------- transpose y1, aft projection, A2A #1 --------------
        y1 = qkv[2]
        with nc.named_scope("proj1", notify=True), \
             tc.tile_pool(name="pC", bufs=1) as pC, \
             tc.tile_pool(name="psC", bufs=2, space="PSUM") as psC:
            y1T = pC.tile([128, 8, 512], F32R, tag="y1T", name="y1T")
            a_sb = pC.tile([128, 4, 1024], F32, tag="bigB", name="a_sb")
            for b in range(2):
                for cb in range(8):
                    pst = psC.tile([128, 256], F32, tag="ptr", name="ptr")
                    for rl in range(2):
                        rt = 2 * b + rl
                        nc.tensor.transpose(
                            pst[:, 128 * rl:128 * (rl + 1)],
                            y1[:, rt, 128 * cb:128 * (cb + 1)], ident[:])
                    nc.any.tensor_copy(y1T[:, cb, 256 * b:256 * b + 256],
                                       pst[:])
                for rl in range(2):
                    rt = 2 * b + rl
                    for oc in range(2):
                        pa = psC.tile([128, 512], F32, tag="pa", name="pa")
                        for cb in range(8):
                            nc.tensor.matmul(
                                pa[:],
                                y1T[:, cb, 256 * b + 128 * rl:
                                    256 * b + 128 * (rl + 1)],
                                projw[:, cb, 512 * oc:512 * (oc + 1)],
                                start=(cb == 0), stop=(cb == 7))
                        nc.any.tensor_copy(
                            a_sb[:, rt, 512 * oc:512 * (oc + 1)], pa[:])
                for d in range(NCORES):
                    nc.sync.dma_start(
                        out=a2a1_in[b][256 * d:256 * (d + 1), :]
                            .rearrange("(rt p) c -> p rt c", p=128),
                        in_=a_sb[:, 2 * b:2 * b + 2, 128 * d:128 * (d + 1)])
                nc.gpsimd.collective_compute(
                    "AllToAll", ALU.bypass,
                    replica_groups=[list(range(NCORES))],
                    ins=[a2a1_in[b][:].opt()], outs=[a2a1_out[b][:].opt()])

        # ---------------- stage 2: SDPA over 2 heads, pipelined per b ----
        with nc.named_scope("sdpa", notify=True), \
             tc.tile_pool(name="pE", bufs=1) as pE, \
             tc.tile_pool(name="psE", bufs=2, space="PSUM") as psE, \
             tc.tile_pool(name="psY", bufs=1, space="PSUM") as psY, \
             tc.tile_pool(name="pe16", bufs=3) as pe16:
            A2 = gp.tile([128, 32, 128], F32, tag="bigA", name="A2")
            V16 = pE.tile([128, 32, 2, 65], F16, tag="V16", name="V16")
            QT = pE.tile([128, 2, 2048], F32R, tag="QT", name="QT")
            YRAW = gp.tile([65, 16, 512], F32, tag="tk", name="YRAW")
            RSR = pE.tile([64, 512], F32, tag="RSR", name="RSR")
            for b in range(2):
                for j in range(16):
                    ro = 256 * (j // 2) + 128 * (j % 2)
                    nc.sync.dma_start(out=A2[:, b * 16 + j, :],
                                      in_=a2a1_out[b][ro:ro + 128, :])
                sq2 = gp.tile([128, 4096], F32, tag="tq", name=f"sq2_{b}")
                bs = slice(16 * b, 16 * (b + 1))
                nc.scalar.activation(
                    sq2[:, :2048],
                    A2[:, bs, :].rearrange("p s c -> p (s c)"), AF.Square)
                ssq2 = pE.tile([128, 32], F32, tag="ssq2", name=f"ssq2_{b}")
                nc.vector.tensor_reduce(
                    ssq2[:],
                    sq2[:, :2048].rearrange("p (s h d) -> p (s h) d",
                                            s=16, h=2),
                    axis=mybir.AxisListType.X, op=ALU.add)
                nc.vector.tensor_scalar(ssq2[:], ssq2[:], 1.0 / 64, EPS,
                                        ALU.mult, ALU.add)
                rf2 = _quake_rsqrt(nc, pE, ssq2[:], 32, f"rf2_{b}")
                rf2b = rf2[:].rearrange("p (s h) -> p s h", s=16) \
                    .unsqueeze(3).broadcast_to([128, 16, 2, 64])
                A24 = A2[:, bs, :].rearrange("p s (h d) -> p s h d", h=2)
                nc.vector.tensor_tensor(A24, A24, rf2b, ALU.mult)

                nc.vector.tensor_copy(V16[:, bs, :, 0:64], A24)
                nc.vector.memset(V16[:, bs, :, 64:65], 1.0)

                x1 = A24[:, :, :, 0:16]
                x2 = A24[:, :, :, 32:48]
                cb2 = rotc2[:].unsqueeze(1).broadcast_to([128, 16, 2, 16])
                sb2 = rots2[:].unsqueeze(1).broadcast_to([128, 16, 2, 16])
                ru1 = pE.tile([128, 16, 2, 16], F32, tag="ru1s",
                              name=f"ru1s{b}")
                ru2 = pE.tile([128, 16, 2, 16], F32, tag="ru2s",
                              name=f"ru2s{b}")
                rt1 = pE.tile([128, 16, 2, 16], F32, tag="rt1s",
                              name=f"rt1s{b}")
                nc.vector.tensor_tensor(ru1[:], x2, sb2, ALU.mult)
                nc.vector.tensor_tensor(ru2[:], x1, sb2, ALU.mult)
                nc.vector.tensor_tensor(rt1[:], x1, cb2, ALU.mult)
                nc.vector.tensor_tensor(x1, rt1[:], ru1[:], ALU.add)
                nc.vector.tensor_tensor(rt1[:], x2, cb2, ALU.mult)
                nc.vector.tensor_tensor(x2, rt1[:], ru2[:], ALU.subtract)

                for jq in range(4):
                    pst2 = psE.tile([128, 512], F32, tag="ptr2",
                                    name="ptr2")
                    for k4 in range(4):
                        j = 4 * jq + k4
                        nc.tensor.transpose(
                            pst2[:, 128 * k4:128 * (k4 + 1)],
                            A2[:, b * 16 + j, :], ident[:])
                    nc.any.tensor_copy(QT[:, b, 512 * jq:512 * (jq + 1)],
                                       pst2[:])

                for qc in range(4):
                    pys = []
                    for h2 in range(2):
                        pyt = psY.tile([65, 512], F32, tag=f"py{h2}",
                                       name=f"py{h2}")
                        pys.append(pyt)
                    nkt = 4 * qc + 4
                    for kt in range(nkt):
                        ps_s = psE.tile([128, 1024], F32, tag="ps_s",
                                        name="ps_s")
                        for hh in range(2):
                            hb = 64 * hh
                            nc.tensor.matmul(
                                ps_s[:, 512 * hh:512 * (hh + 1)],
                                QT[hb:hb + 64, b, 128 * kt:128 * (kt + 1)],
                                QT[hb:hb + 64, b, 512 * qc:512 * (qc + 1)],
                                start=True, stop=True)
                        e16 = pe16.tile([128, 1024], F16, tag="e16",
                                        name="e16")
                        nc.scalar.activation(e16[:], ps_s[:], AF.Exp,
                                             scale=SDPA_SCALE)
                        if kt >= 4 * qc:
                            for hh in range(2):
                                eh = e16[:, 512 * hh:512 * (hh + 1)]
                                nc.gpsimd.affine_select(
                                    out=eh, in_=eh,
                                    compare_op=ALU.is_ge, fill=0.0,
                                    base=512 * qc - 128 * kt,
                                    pattern=[[1, 512]],
                                    channel_multiplier=-1)
                        for hh in range(2):
                            nc.tensor.matmul(
                                pys[hh][:], V16[:, b * 16 + kt, hh, :],
                                e16[:, 512 * hh:512 * (hh + 1)],
                                start=(kt == 0), stop=(kt == nkt - 1))
                    for hh in range(2):
                        slot = (b * 2 + hh) * 4 + qc
                        nc.any.tensor_copy(YRAW[:, slot, :], pys[hh][:])

                # per-b softmax denominators + normalize + ship
                bslots = slice(8 * b, 8 * (b + 1))
                rs_b = dpool.tile([8, 512], F32, name=f"rs_b{b}")
                nc.sync.dma_start(out=rs_b[:], in_=YRAW[64:65, bslots, :])
                RSS = pE.tile([8, 512], F32, tag="RSS", name=f"RSS{b}")
                nc.sync.dma_start(out=RSS[:], in_=rs_b[:])
                nc.vector.reciprocal(RSS[:], RSS[:])
                rs_b2 = dpool.tile([8, 512], F32, name=f"rs_b2{b}")
                nc.sync.dma_start(out=rs_b2[:], in_=RSS[:])
                RSS1 = pE.tile([1, 8, 512], F32, tag="RSS1", name=f"RSS1{b}")
                nc.sync.dma_start(out=RSS1[:], in_=rs_b2[:])
                for sl in range(8):
                    slot = 8 * b + sl
                    nc.gpsimd.partition_broadcast(RSR[:],
                                                  RSS1[0:1, sl, :])
                    nc.vector.tensor_tensor(
                        YRAW[0:64, slot, :].bitcast(F32R),
                        YRAW[0:64, slot, :], RSR[:], ALU.mult)
                for sl in range(8):
                    slot = 8 * b + sl
                    hh, qc = sl // 4, sl % 4
                    for half in range(2):
                        d = 2 * qc + half
                        nc.sync.dma_start(
                            out=a2a2_in[b][128 * d + 64 * hh:
                                           128 * d + 64 * hh + 64, :],
                            in_=YRAW[0:64, slot,
                                     256 * half:256 * half + 256])
                nc.gpsimd.collective_compute(
                    "AllToAll", ALU.bypass,
                    replica_groups=[list(range(NCORES))],
                    ins=[a2a2_in[b][:].opt()],
                    outs=[a2a2_out[b][:].opt()])

        # ---------------- stage 3: output projection ---------------------
        with nc.named_scope("mha", notify=True), \
             tc.tile_pool(name="pH", bufs=1) as pH, \
             tc.tile_pool(name="psH", bufs=2, space="PSUM") as psH:
            nc.sync.dma_start(out=projw[:], in_=mhaT_d[:].bitcast(F32R))
            out_sb = pH.tile([128, 4, 1024], F32, tag="out_sb",
                             name="out_sb")
            for b in range(2):
                YF = pH.tile([128, 8, 256], F32R, tag="YF", name=f"YF{b}")
                for s_ in range(NCORES):
                    nc.sync.dma_start(
                        out=YF[:, s_, :],
                        in_=a2a2_out[b][128 * s_:128 * (s_ + 1), :]
                        .bitcast(F32R))
                for rl in range(2):
                    rt = 2 * b + rl
                    for oc in range(2):
                        pm = psH.tile([128, 512], F32, tag="pm", name="pm")
                        for s_ in range(NCORES):
                            nc.tensor.matmul(
                                pm[:],
                                YF[:, s_, 128 * rl:128 * (rl + 1)],
                                projw[:, s_, 512 * oc:512 * (oc + 1)],
                                start=(s_ == 0), stop=(s_ == 7))
                        nc.any.tensor_copy(
                            out_sb[:, rt, 512 * oc:512 * (oc + 1)], pm[:])
            nc.sync.dma_start(out=out_d[:], in_=out_sb[:])

    nc.compile()
    return nc


def _host_inputs(x, kqv, c_proj):
    """Build per-core input maps from the full problem inputs."""
    A = kqv[:, :5].astype(np.float64)     # sin coefs (col n)
    Bc = kqv[:, 5:].astype(np.float64)    # cos coefs
    coef = np.zeros((8, 3), np.float64)   # basis {s,c1,m,p,sp,cp,mp,mm}
    K = A[:, 0] + Bc[:, 0] + Bc[:, 2] + Bc[:, 4]
    coef[0] = A[:, 1] + 3.0 * A[:, 3]
    coef[1] = Bc[:, 1] + Bc[:, 3]
    coef[2] = 2.0 * A[:, 2] + 4.0 * A[:, 4]
    coef[3] = -2.0 * Bc[:, 2]
    coef[4] = -4.0 * A[:, 3]
    coef[5] = -4.0 * Bc[:, 3]
    coef[6] = -8.0 * A[:, 4]
    coef[7] = -8.0 * Bc[:, 4]

    eye = np.eye(128, dtype=np.float32)
    combw = np.zeros((128, 24, 128), np.float32)
    for i in range(3):
        for f in range(8):
            combw[:, 8 * i + f, :] = eye * np.float32(coef[f, i])
    kvec = np.zeros((128, 4), np.float32)
    kvec[:, :3] = K.astype(np.float32)[None, :]

    freq = (1.0 / 1024.0) ** np.linspace(0.0, 1.0, 16, dtype=np.float32)
    hh = np.arange(16, dtype=np.float32)
    theta = np.outer(hh, freq)                       # (16 heads, 16 j)
    rotc1 = np.broadcast_to(np.cos(theta), (128, 16, 16)).copy()
    rots1 = np.broadcast_to(np.sin(theta), (128, 16, 16)).copy()

    W1 = c_proj[:, :C]
    W2 = c_proj[:, C:]
    aftT = (0.5 * W1.T).reshape(8, 128, 1024).transpose(1, 0, 2).copy()
    mhaT = W2.T.reshape(8, 128, 1024).transpose(1, 0, 2).copy()

    in_maps = []
    for c in range(NCORES):
        xs = x[:, TSH * c:TSH * (c + 1), :].reshape(4, 128, 1024) \
            .transpose(1, 0, 2).copy()
        th2 = theta[2 * c:2 * c + 2, :]
        rotc2 = np.broadcast_to(np.cos(th2), (128, 2, 16)).copy()
        rots2 = np.broadcast_to(np.sin(th2), (128, 2, 16)).copy()
        in_maps.append(dict(xs=np.ascontiguousarray(xs), combw=combw,
                            kvec=kvec, rotc1=rotc1, rots1=rots1,
                            rotc2=rotc2, rots2=rots2, aftT=aftT,
                            mhaT=mhaT))
    return in_maps


def kernel(x, kqv, c_proj):
    x = np.asarray(x, np.float32)
    kqv = np.asarray(kqv, np.float32)
    c_proj = np.asarray(c_proj, np.float32)
    if "nc" not in _CACHE:
        _CACHE["nc"] = build()
    nc = _CACHE["nc"]
    in_maps = _host_inputs(x, kqv, c_proj)
    res = run_bass_kernel_spmd(nc, in_maps, core_ids=list(range(NCORES)))
    out = np.empty((B, T, C), np.float32)
    for c in range(NCORES):
        oc = res.results[c]["out"]          # (128, 4, 1024)
        oc = oc.transpose(1, 0, 2).reshape(B, TSH, C)
        out[:, TSH * c:TSH * (c + 1), :] = oc
    return out
